# revision 14
# baseline (speedup 1.0000x reference)
"""Trainium2 Bass kernel for nn_DCRNN_Layer (gnn_message_passing).

Strategy (v2)
-------------
The reference graph is deterministic: node i has out-edges to (i+1..i+16) mod N,
so each Chebyshev diffusion step is a banded circulant operator.  Level-1
propagation is applied as dense 128x128 {0,1} band-block matmuls on the
TensorEngine.  Level-2 propagation uses HOST-PRECOMPUTED composite band
matrices M2 = B*diag(norm)*B (bandwidth 32, same-tile block + 32-row corner
block), replacing a two-hop (q1 + norm-scaled-band) scheme: this removes
~8 LDWEIGHTS + 512 matmul columns + 2 PSUM->SBUF casts per node tile.

Gates are computed "feature first":
    t1 = Xc@W0' + LF(f1) + LR(r1) + M2F(f2) + M2R(r2) + corner blocks
with f1 = (no*Xc)@W1, f2 = (no*Xc)@(2*W2), etc.  Corner blocks only carry
<=32 nonzero source rows and are issued as 32-row stationaries.

Elementwise work is batched over tile PAIRS (one sigmoid for z+r of two
tiles, one cast per pair, paired final combine) to amortize per-instruction
engine overheads.  The three R-scaled stationaries [X|H*R] live interleaved
in one xhr3 tensor so each tile's H-half update is a single broadcast
multiply against the transposed sigmoid(R).

Sharding: 8 cores = batch(4) x node-halves(2).  Each core gets a 5376-node
extended shard (64-node halo each side, circularly wrapped on the host) so no
inter-core communication is needed.  Matmul operands are fp16 (PSUM fp32).

kernel() is self-contained: host preprocessing (numpy) -> 8-core SPMD Bass
kernel -> host unshard.  If the edge structure deviates from the banded
pattern (or biases are nonzero), a numpy fallback reproduces the reference.
"""

import numpy as np
from contextlib import ExitStack

# Problem constants (hardcoded per contract).
B, N, DEG, IN, OUT, K = 4, 10000, 16, 64, 64, 3
C = IN + OUT          # 128
EPS = 1e-8
NCORES = 8
HALF = N // 2         # 5000 nodes per node-shard
T = 42                # node tiles per extended shard
N2 = T * 128          # 5376 extended shard nodes
LHALO = 64            # left halo (ext index of output node 0)

_prog_cache = {}


# ----------------------------------------------------------------------------
# Device program
# ----------------------------------------------------------------------------
def _build_program(n_tiles=T):
    import concourse.bass as bass
    import concourse.bacc as bacc
    import concourse.tile as tile
    from concourse import mybir

    f16 = mybir.dt.float16
    f32 = mybir.dt.float32
    Tn = n_tiles
    NP = Tn // 2

    nc = bacc.Bacc(
        "TRN2",
        target_bir_lowering=False,
        debug=False,
        enable_asserts=False,
        num_devices=NCORES,
    )

    din = {}
    for name, shape, dt in [
        ("xnn", [128, Tn, 2, 128], f16),
        ("m2p", [128, Tn, 4, 128], f16),
        ("xhr3i", [128, Tn, 3, 128], f16),
        ("hext", [128, Tn, 64], f16),
        ("wzr", [128, 640], f16),
        ("wh", [128, 320], f16),
        ("bands", [128, 512], f16),
        ("ident", [128, 128], f16),
    ]:
        din[name] = nc.dram_tensor(name, shape, dt, kind="ExternalInput").ap()
    d_out = nc.dram_tensor("out", [128, Tn, 64], f16, kind="ExternalOutput").ap()

    SIG = mybir.ActivationFunctionType.Sigmoid
    TANH = mybir.ActivationFunctionType.Tanh

    with tile.TileContext(nc) as tc:
        with ExitStack() as ctx:
            consts = ctx.enter_context(tc.tile_pool(name="consts", bufs=1))

            def load_small(name):
                t = consts.tile(din[name].shape, din[name].dtype, tag=name)
                nc.sync.dma_start(t[:], din[name][:])
                return t

            wzr = load_small("wzr")
            wh = load_small("wh")
            bands = load_small("bands")
            ident = load_small("ident")

            # Host-packed tensors.  All loads dispatched upfront, split 16
            # ways across the DMA queues; dispatch cost is paid on engines
            # whose first compute use comes iterations later.
            xnn = consts.tile([128, Tn, 2, 128], f16, tag="xnn")
            m2p = consts.tile([128, Tn, 4, 128], f16, tag="m2p")
            xhr3 = consts.tile([128, Tn, 3, 128], f16, tag="xhr3")
            hext = consts.tile([128, Tn, 64], f16, tag="hext")

            # front-loaded split schedule: tiny leading windows so tile-0
            # operands arrive within a few us, coarse tail windows
            SPLITS = [(0, 2), (2, 4), (4, 6), (6, 9), (9, 12), (12, 16),
                      (16, 20), (20, 25), (25, 30), (30, 36), (36, 42)]

            for a, b in SPLITS:
                nc.scalar.dma_start(xnn[:, a:b], din["xnn"][:, a:b])
            for a, b in SPLITS:
                nc.sync.dma_start(m2p[:, a:b, 0:2], din["m2p"][:, a:b, 0:2])
            for a, b in SPLITS:
                nc.sync.dma_start(m2p[:, a:b, 2:4], din["m2p"][:, a:b, 2:4])
            for a, b in SPLITS:
                nc.gpsimd.dma_start(xhr3[:, a:b], din["xhr3i"][:, a:b])
            for a, b in ((0, 10), (10, 21), (21, 32), (32, 42)):
                nc.gpsimd.dma_start(hext[:, a:b], din["hext"][:, a:b])

            outst = consts.tile([128, Tn, 64], f16, tag="outst")

            scbuf = consts.tile([128, Tn, 4, 128], f16, tag="scbuf")
            schbuf = consts.tile([128, Tn, 4, 64], f16, tag="schbuf")
            s1_pool = ctx.enter_context(tc.tile_pool(name="s1", bufs=11))
            rts_pool = ctx.enter_context(tc.tile_pool(name="rts", bufs=3))
            fin_pool = ctx.enter_context(tc.tile_pool(name="fin", bufs=6))

            py = ctx.enter_context(tc.tile_pool(name="py", bufs=2, space="PSUM"))
            pt1 = ctx.enter_context(tc.tile_pool(name="pt1", bufs=2, space="PSUM"))
            pyh = ctx.enter_context(tc.tile_pool(name="pyh", bufs=2, space="PSUM"))
            pth = ctx.enter_context(tc.tile_pool(name="pth", bufs=2, space="PSUM"))

            s1l = [None] * NP         # per-pair [128, 2, 2, 64] f16
            t1l = [None] * NP
            thl = [None] * NP

            mm = nc.tensor.matmul

            # scbuf groups: 0=f1, 1=f2, 2=r1, 3=r2 (each 128 = z|r gates)
            # schbuf groups: 0=fh1, 1=fh2, 2=rh1, 3=rh2 (each 64)
            # bands cols: LF 0:128, UF 128:256, LR 256:384, UR 384:512
            # m2p slots: 0=M2F_same, 1=M2R_same, 2=corners (fwd rows 64:128,
            #            rev rows 0:32)
            def accum_list(out_pair, buf, p, w0_movs):
                """Fused pair accumulation matmul list (z/r or h path)."""
                a, b = 2 * p, 2 * p + 1
                ms = [(out_pair[:], bands[:, 0:128], buf[:, a:b + 1, 0, :]),
                      (out_pair[:], bands[:, 256:384], buf[:, a:b + 1, 2, :])]
                if p > 0:
                    ms.append((out_pair[:], bands[:, 128:256],
                               buf[:, a - 1:b, 0, :]))
                else:
                    ms.append((out_pair[:, 1], bands[:, 128:256],
                               buf[:, a:a + 1, 0, :]))
                if p < NP - 1:
                    ms.append((out_pair[:], bands[:, 384:512],
                               buf[:, b:b + 2, 2, :]))
                else:
                    ms.append((out_pair[:, 0], bands[:, 384:512],
                               buf[:, b:b + 1, 2, :]))
                for t, j in ((0, a), (1, b)):
                    ms.append((out_pair[:, t], xhr3[:, j, 0, :], w0_movs))
                    ms.append((out_pair[:, t], m2p[:, j, 0, :], buf[:, j, 1, :]))
                    ms.append((out_pair[:, t], m2p[:, j, 1, :], buf[:, j, 3, :]))
                    if j > 0:
                        ms.append((out_pair[:, t], m2p[:, j, 2, :],
                                   buf[:, j - 1, 1, :]))
                    if j < Tn - 1:
                        ms.append((out_pair[:, t], m2p[:, j, 3, :],
                                   buf[:, j + 1, 3, :]))
                return ms

            def emit_interleaved(la, lb):
                # Alternate between the two accumulation groups (different
                # PSUM banks) so consecutive PE instructions never chain on
                # the same accumulator region.
                na, nb = len(la), len(lb)
                for i in range(max(na, nb)):
                    if i < na:
                        dst, st, mv = la[i]
                        mm(dst, st, mv, start=(i == 0), stop=(i == na - 1))
                    if i < nb:
                        dst, st, mv = lb[i]
                        mm(dst, st, mv, start=(i == 0), stop=(i == nb - 1))

            # Pipeline phases (pair index = it - offset):
            #   A: z/r features        C1: sigmoid        D: h features
            #   B: z/r accumulation    C2: R transpose+    E: h accumulation
            #                              xhr3 update     F: combine + out
            for it in range(NP + 13):
                # ---- A: z/r feature matmuls for pair `it` -----------------
                if it < NP:
                    for t in (0, 1):
                        j = 2 * it + t
                        y = py.tile([128, 4, 128], f32, tag="y")
                        mm(y[:, 0:2, :], xnn[:, j, 0, :], wzr[:, 128:384],
                           start=True, stop=False)
                        mm(y[:, 2:4, :], xnn[:, j, 1, :], wzr[:, 384:640],
                           start=False, stop=True)
                        nc.vector.tensor_copy(scbuf[:, j], y[:])

                # ---- C2: R transpose + xhr3 H-half update for pair it-6 ---
                if 6 <= it <= NP + 5:
                    p = it - 6
                    ss = s1l[p]
                    tp = pt1.tile([128, 2, 128], f16, tag="t1")
                    mm(tp[:, 0, :], ss[:, 0], ident[:],
                       is_transpose=True, start=True, stop=False)
                    mm(tp[:, 1, :], ss[:, 1], ident[:],
                       is_transpose=True, start=False, stop=True)
                    rts = rts_pool.tile([128, 2, 128], f16, tag="rts")
                    nc.scalar.activation(rts[64:128], tp[64:128],
                                         mybir.ActivationFunctionType.Copy)
                    a = 2 * p
                    rb = rts[64:128, :, None, :].broadcast_to([64, 2, 3, 128])
                    nc.gpsimd.tensor_mul(xhr3[64:128, a:a + 2],
                                         xhr3[64:128, a:a + 2], rb)

                # ---- B+E: z/r accumulation (pair it-3) interleaved with
                #      h accumulation (pair it-12) ---------------------------
                msB, msE, t1 = [], [], None
                if 3 <= it <= NP + 2:
                    p = it - 3
                    t1 = pt1.tile([128, 2, 2, 64], f32, tag="t1")
                    t1l[p] = t1
                    msB = accum_list(t1, scbuf, p, wzr[:, 0:128])
                if 11 <= it <= NP + 10:
                    p = it - 11
                    th = pth.tile([128, 2, 1, 64], f32, tag="th")
                    thl[p] = th
                    msE = accum_list(th, schbuf, p, wh[:, 0:64])
                emit_interleaved(msB, msE)

                # ---- C1: gate sigmoids for pair it-4 (straight from PSUM) -
                if 4 <= it <= NP + 3:
                    p = it - 4
                    ss = s1_pool.tile([128, 2, 2, 64], f16, tag="s1")
                    nc.scalar.activation(ss[:], t1l[p][:], SIG)
                    s1l[p] = ss

                # ---- D: h feature matmuls for pair it-8 -------------------
                if 8 <= it <= NP + 7:
                    p = it - 8
                    yh = pyh.tile([128, 2, 4, 64], f32, tag="yh")
                    for t in (0, 1):
                        j = 2 * p + t
                        mm(yh[:, t, 0:2, :], xhr3[:, j, 1, :], wh[:, 64:192],
                           start=(t == 0), stop=False)
                        mm(yh[:, t, 2:4, :], xhr3[:, j, 2, :], wh[:, 192:320],
                           start=False, stop=(t == 1))
                    nc.vector.tensor_copy(schbuf[:, 2 * p:2 * p + 2], yh[:])


                # ---- F: H_tilde + final combine for pair it-12 ------------
                if 12 <= it <= NP + 11:
                    p = it - 12
                    ht = fin_pool.tile([128, 2, 64], f32, tag="ht")
                    nc.scalar.activation(ht[:], thl[p][:, :, 0, :], TANH)
                    d = fin_pool.tile([128, 2, 64], f32, tag="fd")
                    nc.gpsimd.tensor_sub(d[:], hext[:, 2 * p:2 * p + 2, :],
                                         ht[:])
                    e = fin_pool.tile([128, 2, 64], f32, tag="fe")
                    nc.vector.tensor_mul(e[:], d[:], s1l[p][:, :, 0, :])
                    nc.gpsimd.tensor_add(outst[:, 2 * p:2 * p + 2, :], e[:],
                                         ht[:])
                    if (p + 1) % 3 == 0 or p == NP - 1:
                        o0 = (p // 3) * 6
                        o1 = 2 * (p + 1)
                        nc.sync.dma_start(d_out[:, o0:o1], outst[:, o0:o1])

    nc.compile()
    return nc


def _get_program(n_tiles=T):
    if n_tiles not in _prog_cache:
        _prog_cache[n_tiles] = _build_program(n_tiles)
    return _prog_cache[n_tiles]


# ----------------------------------------------------------------------------
# Host-side preparation
# ----------------------------------------------------------------------------
def _band_matrices():
    s = np.arange(128)[:, None]
    d = np.arange(128)[None, :]
    bf_l = ((d - s >= 1) & (d - s <= DEG)).astype(np.float16)
    bf_u = (s - d >= 128 - DEG).astype(np.float16)
    br_l = ((s - d >= 1) & (s - d <= DEG)).astype(np.float16)
    br_u = (d - s >= 128 - DEG).astype(np.float16)
    return np.concatenate([bf_l, bf_u, br_l, br_u], axis=1)  # [128, 512]


def _m2_blocks(no_e, ni_e):
    """Composite level-2 band stationaries, [s, d] layout, per tile.

    M2F_g[d, s] = sum_{m=max(s+1, d-16)}^{min(s+16, d-1)} no[m]  (d-s in [2,32])
    M2R_g[d, s] = sum_{m=max(d+1, s-16)}^{min(d+16, s-1)} ni[m]  (s-d in [2,32])
    Returns [T, 4, 128, 128] f16: M2F_same, M2R_same, M2F_corner (prev
    tile sources, zero-padded to full rows), M2R_corner (next tile).
    """
    n2 = len(no_e)
    Cno = np.concatenate([[0.0], np.cumsum(no_e, dtype=np.float64)])
    Cni = np.concatenate([[0.0], np.cumsum(ni_e, dtype=np.float64)])

    def block(Cm, d, s, rev):
        if rev:
            lo = np.maximum(d + 1, s - 16)
            hi = np.minimum(d + 16, s - 1)
            off = s - d
        else:
            lo = np.maximum(s + 1, d - 16)
            hi = np.minimum(s + 16, d - 1)
            off = d - s
        valid = ((off >= 2) & (off <= 32) & (s >= 0) & (s < n2)
                 & (d >= 0) & (d < n2) & (hi >= lo))
        lo_c = np.clip(lo, 0, n2)
        hi_c = np.clip(hi + 1, 0, n2)
        out = Cm[np.maximum(hi_c, lo_c)] - Cm[lo_c]
        return np.where(valid, out, 0.0).astype(np.float32)

    nt = n2 // 128
    sl = np.arange(128)
    arr = np.zeros((nt, 4, 128, 128), np.float32)
    for t in range(nt):
        t0 = t * 128
        dg = t0 + sl[None, :]           # dest cols
        sg = t0 + sl[:, None]           # source rows (same tile)
        arr[t, 0] = block(Cno, dg, sg, rev=False)
        arr[t, 1] = block(Cni, dg, sg, rev=True)
        if t > 0:
            arr[t, 2] = block(Cno, dg, t0 - 128 + sl[:, None], rev=False)
        if t < nt - 1:
            arr[t, 3] = block(Cni, dg, t0 + 128 + sl[:, None], rev=True)
    return arr.astype(np.float16)


def _pack_weights_zr(Wz, Wr):
    def parts(W):
        w0 = W[0, 0] + W[1, 0] - W[0, 2] - W[1, 2]
        return w0, W[0, 1], 2.0 * W[0, 2], W[1, 1], 2.0 * W[1, 2]
    w0z, f1z, f2z, r1z, r2z = parts(Wz)
    w0r, f1r, f2r, r1r, r2r = parts(Wr)
    return np.concatenate(
        [w0z, w0r, f1z, f1r, f2z, f2r, r1z, r1r, r2z, r2r], axis=1
    ).astype(np.float16)  # [128, 640]


def _pack_weights_h(Wh):
    w0 = Wh[0, 0] + Wh[1, 0] - Wh[0, 2] - Wh[1, 2]
    return np.concatenate(
        [w0, Wh[0, 1], 2.0 * Wh[0, 2], Wh[1, 1], 2.0 * Wh[1, 2]], axis=1
    ).astype(np.float16)  # [128, 320]


def _is_banded_graph(row, col):
    if row.shape != (N * DEG,):
        return False
    r_exp = np.repeat(np.arange(N, dtype=np.int64), DEG)
    if not np.array_equal(row, r_exp):
        return False
    c_exp = (r_exp + np.tile(np.arange(1, DEG + 1, dtype=np.int64), N)) % N
    return np.array_equal(col, c_exp)


def _numpy_fallback(X, H, edge_weight, Wz, bz, Wr, br, Wh, bh, row, col):
    """Exact reference math on the host (only used if inputs deviate)."""
    deg_out = np.bincount(row, weights=edge_weight, minlength=N).astype(np.float32)
    deg_in = np.bincount(col, weights=edge_weight, minlength=N).astype(np.float32)
    norm_out = (1.0 / (deg_out + EPS))[row].astype(np.float32)
    norm_in = (1.0 / (deg_in + EPS))[col].astype(np.float32)

    def prop(x, src, dst, nrm):
        msg = x[:, src, :] * nrm[None, :, None]
        out = np.zeros_like(x)
        np.add.at(out, (slice(None), dst), msg)
        return out

    def dconv(Xc, W, b):
        Hh = Xc @ (W[0, 0] + W[1, 0])
        t1o = prop(Xc, row, col, norm_out)
        t1i = prop(Xc, col, row, norm_in)
        Hh = Hh + t1o @ W[0, 1] + t1i @ W[1, 1]
        for k in range(2, K):
            t1o = 2.0 * prop(t1o, row, col, norm_out) - Xc
            t1i = 2.0 * prop(t1i, col, row, norm_in) - Xc
            Hh = Hh + t1o @ W[0, k] + t1i @ W[1, k]
        return Hh + b

    XH = np.concatenate([X, H], axis=-1)
    Z = 1.0 / (1.0 + np.exp(-dconv(XH, Wz, bz)))
    R = 1.0 / (1.0 + np.exp(-dconv(XH, Wr, br)))
    XHR = np.concatenate([X, H * R], axis=-1)
    Ht = np.tanh(dconv(XHR, Wh, bh))
    return (Z * H + (1.0 - Z) * Ht).astype(np.float32)


def make_in_maps(X, H, edge_weight, Wz, bz, Wr, br, Wh, bh, row, col):
    """Build the 8 per-core input dicts (host sharding + preprocessing)."""
    deg_out = np.bincount(row, weights=edge_weight, minlength=N).astype(np.float32)
    deg_in = np.bincount(col, weights=edge_weight, minlength=N).astype(np.float32)
    n_out = (1.0 / (deg_out + EPS)).astype(np.float32)
    n_in = (1.0 / (deg_in + EPS)).astype(np.float32)

    shared = {
        "wzr": _pack_weights_zr(Wz, Wr),
        "wh": _pack_weights_h(Wh),
        "bands": _band_matrices(),
        "ident": np.eye(128, dtype=np.float16),
    }

    in_maps = []
    for core in range(NCORES):
        b, half = core // 2, core % 2
        g0 = half * HALF - LHALO
        idx = (g0 + np.arange(N2)) % N
        ext = np.concatenate([X[b], H[b]], axis=1)[idx]          # [N2, 128] f32
        no_e = n_out[idx]
        ni_e = n_in[idx]
        m = dict(shared)
        xht = ext.T.astype(np.float16).reshape(128, T, 128)
        xhno = (ext * no_e[:, None]).T.astype(np.float16).reshape(128, T, 128)
        xhni = (ext * ni_e[:, None]).T.astype(np.float16).reshape(128, T, 128)
        m["xnn"] = np.ascontiguousarray(
            np.stack([xhno, xhni], axis=2))                  # [128, T, 2, 128]
        m["xhr3i"] = np.ascontiguousarray(
            np.stack([xht, xhno, xhni], axis=2))             # [128, T, 3, 128]
        m["hext"] = np.ascontiguousarray(
            H[b][idx].reshape(T, 128, 64).transpose(1, 0, 2)).astype(np.float16)
        m["m2p"] = np.ascontiguousarray(
            _m2_blocks(no_e, ni_e).transpose(2, 0, 1, 3))
        in_maps.append(m)
    return in_maps


def unshard_outputs(results):
    out = np.empty((B, N, OUT), dtype=np.float32)
    for core in range(NCORES):
        b, half = core // 2, core % 2
        res = results[core]["out"].astype(np.float32).reshape(128, T, 64)
        ext = res.transpose(1, 0, 2).reshape(N2, 64)
        out[b, half * HALF:(half + 1) * HALF] = ext[LHALO:LHALO + HALF]
    return out


def kernel(X, H, edge_weight, Wz, bz, Wr, br, Wh, bh, edge_index):
    X = np.asarray(X, dtype=np.float32)
    H = np.asarray(H, dtype=np.float32)
    edge_weight = np.asarray(edge_weight, dtype=np.float32)
    Wz = np.asarray(Wz, dtype=np.float32)
    Wr = np.asarray(Wr, dtype=np.float32)
    Wh = np.asarray(Wh, dtype=np.float32)
    bz = np.asarray(bz, dtype=np.float32)
    br = np.asarray(br, dtype=np.float32)
    bh = np.asarray(bh, dtype=np.float32)
    ei = np.asarray(edge_index)
    row = ei[0].astype(np.int64)
    col = ei[1].astype(np.int64)

    if (not _is_banded_graph(row, col) or np.any(bz) or np.any(br)
            or np.any(bh)):
        return _numpy_fallback(X, H, edge_weight, Wz, bz, Wr, br, Wh, bh,
                               row, col)

    from concourse import bass_utils

    nc = _get_program(T)
    in_maps = make_in_maps(X, H, edge_weight, Wz, bz, Wr, br, Wh, bh, row, col)
    res = bass_utils.run_bass_kernel_spmd(nc, in_maps, list(range(NCORES)))
    return unshard_outputs(res.results)


# revision 15
# speedup vs baseline: 1.1465x; 1.1465x over previous
"""Trainium2 Bass kernel for nn_DCRNN_Layer (gnn_message_passing).

Strategy (v2)
-------------
The reference graph is deterministic: node i has out-edges to (i+1..i+16) mod N,
so each Chebyshev diffusion step is a banded circulant operator.  Level-1
propagation is applied as dense 128x128 {0,1} band-block matmuls on the
TensorEngine.  Level-2 propagation uses HOST-PRECOMPUTED composite band
matrices M2 = B*diag(norm)*B (bandwidth 32, same-tile block + 32-row corner
block), replacing a two-hop (q1 + norm-scaled-band) scheme: this removes
~8 LDWEIGHTS + 512 matmul columns + 2 PSUM->SBUF casts per node tile.

Gates are computed "feature first":
    t1 = Xc@W0' + LF(f1) + LR(r1) + M2F(f2) + M2R(r2) + corner blocks
with f1 = (no*Xc)@W1, f2 = (no*Xc)@(2*W2), etc.  Corner blocks only carry
<=32 nonzero source rows and are issued as 32-row stationaries.

Elementwise work is batched over tile PAIRS (one sigmoid for z+r of two
tiles, one cast per pair, paired final combine) to amortize per-instruction
engine overheads.  The three R-scaled stationaries [X|H*R] live interleaved
in one xhr3 tensor so each tile's H-half update is a single broadcast
multiply against the transposed sigmoid(R).

Sharding: 8 cores = batch(4) x node-halves(2).  Each core gets a 5376-node
extended shard (64-node halo each side, circularly wrapped on the host) so no
inter-core communication is needed.  Matmul operands are fp16 (PSUM fp32).

kernel() is self-contained: host preprocessing (numpy) -> 8-core SPMD Bass
kernel -> host unshard.  If the edge structure deviates from the banded
pattern (or biases are nonzero), a numpy fallback reproduces the reference.
"""

import numpy as np
from contextlib import ExitStack

# Problem constants (hardcoded per contract).
B, N, DEG, IN, OUT, K = 4, 10000, 16, 64, 64, 3
C = IN + OUT          # 128
EPS = 1e-8
NCORES = 8
HALF = N // 2         # 5000 nodes per node-shard
T = 42                # node tiles per extended shard
N2 = T * 128          # 5376 extended shard nodes
LHALO = 64            # left halo (ext index of output node 0)

_prog_cache = {}


# ----------------------------------------------------------------------------
# Device program
# ----------------------------------------------------------------------------
def _build_program(n_tiles=T):
    import concourse.bass as bass
    import concourse.bacc as bacc
    import concourse.tile as tile
    from concourse import mybir

    f16 = mybir.dt.float16
    f32 = mybir.dt.float32
    Tn = n_tiles
    NP = Tn // 2

    nc = bacc.Bacc(
        "TRN2",
        target_bir_lowering=False,
        debug=False,
        enable_asserts=False,
        num_devices=NCORES,
    )

    din = {}
    for name, shape, dt in [
        ("xnn", [128, Tn, 2, 128], f16),
        ("m2p", [128, Tn, 4, 128], f16),
        ("xhr3i", [128, Tn, 3, 128], f16),
        ("hext", [128, Tn, 64], f16),
        ("wzr", [128, 640], f16),
        ("wh", [128, 320], f16),
        ("bands", [128, 512], f16),
        ("ident", [128, 128], f16),
    ]:
        din[name] = nc.dram_tensor(name, shape, dt, kind="ExternalInput").ap()
    d_out = nc.dram_tensor("out", [128, Tn, 64], f16, kind="ExternalOutput").ap()

    SIG = mybir.ActivationFunctionType.Sigmoid
    TANH = mybir.ActivationFunctionType.Tanh

    with tile.TileContext(nc) as tc:
        with ExitStack() as ctx:
            consts = ctx.enter_context(tc.tile_pool(name="consts", bufs=1))

            def load_small(name):
                t = consts.tile(din[name].shape, din[name].dtype, tag=name)
                nc.sync.dma_start(t[:], din[name][:])
                return t

            wzr = load_small("wzr")
            wh = load_small("wh")
            bands = load_small("bands")
            ident = load_small("ident")

            # Host-packed tensors.  All loads dispatched upfront, split 16
            # ways across the DMA queues; dispatch cost is paid on engines
            # whose first compute use comes iterations later.
            xnn = consts.tile([128, Tn, 2, 128], f16, tag="xnn")
            m2p = consts.tile([128, Tn, 4, 128], f16, tag="m2p")
            xhr3 = consts.tile([128, Tn, 3, 128], f16, tag="xhr3")
            hext = consts.tile([128, Tn, 64], f16, tag="hext")

            def splits(k):
                step = -(-Tn // k)
                return [(a, min(a + step, Tn)) for a in range(0, Tn, step)]

            for a, b in splits(16):
                nc.scalar.dma_start(xnn[:, a:b], din["xnn"][:, a:b])
            for a, b in splits(16):
                nc.sync.dma_start(m2p[:, a:b], din["m2p"][:, a:b])
            for a, b in splits(16):
                nc.gpsimd.dma_start(xhr3[:, a:b], din["xhr3i"][:, a:b])
            for a, b in splits(4):
                nc.gpsimd.dma_start(hext[:, a:b], din["hext"][:, a:b])

            outst = consts.tile([128, Tn, 64], f16, tag="outst")

            scbuf = consts.tile([128, Tn, 4, 128], f16, tag="scbuf")
            schbuf = consts.tile([128, Tn, 4, 64], f16, tag="schbuf")
            s1_pool = ctx.enter_context(tc.tile_pool(name="s1", bufs=11))
            rts_pool = ctx.enter_context(tc.tile_pool(name="rts", bufs=3))
            fin_pool = ctx.enter_context(tc.tile_pool(name="fin", bufs=6))

            py = ctx.enter_context(tc.tile_pool(name="py", bufs=2, space="PSUM"))
            pt1 = ctx.enter_context(tc.tile_pool(name="pt1", bufs=2, space="PSUM"))
            pyh = ctx.enter_context(tc.tile_pool(name="pyh", bufs=2, space="PSUM"))
            pth = ctx.enter_context(tc.tile_pool(name="pth", bufs=2, space="PSUM"))

            s1l = [None] * NP         # per-pair [128, 2, 2, 64] f16
            t1l = [None] * NP
            thl = [None] * NP

            mm = nc.tensor.matmul

            # scbuf groups: 0=f1, 1=f2, 2=r1, 3=r2 (each 128 = z|r gates)
            # schbuf groups: 0=fh1, 1=fh2, 2=rh1, 3=rh2 (each 64)
            # bands cols: LF 0:128, UF 128:256, LR 256:384, UR 384:512
            # m2p slots: 0=M2F_same, 1=M2R_same, 2=corners (fwd rows 64:128,
            #            rev rows 0:32)
            def accum_list(out_pair, buf, p, w0_movs):
                """Fused pair accumulation matmul list (z/r or h path)."""
                a, b = 2 * p, 2 * p + 1
                ms = [(out_pair[:], bands[:, 0:128], buf[:, a:b + 1, 0, :]),
                      (out_pair[:], bands[:, 256:384], buf[:, a:b + 1, 2, :])]
                if p > 0:
                    ms.append((out_pair[:], bands[:, 128:256],
                               buf[:, a - 1:b, 0, :]))
                else:
                    ms.append((out_pair[:, 1], bands[:, 128:256],
                               buf[:, a:a + 1, 0, :]))
                if p < NP - 1:
                    ms.append((out_pair[:], bands[:, 384:512],
                               buf[:, b:b + 2, 2, :]))
                else:
                    ms.append((out_pair[:, 0], bands[:, 384:512],
                               buf[:, b:b + 1, 2, :]))
                for t, j in ((0, a), (1, b)):
                    ms.append((out_pair[:, t], xhr3[:, j, 0, :], w0_movs))
                    ms.append((out_pair[:, t], m2p[:, j, 0, :], buf[:, j, 1, :]))
                    ms.append((out_pair[:, t], m2p[:, j, 1, :], buf[:, j, 3, :]))
                    if j > 0:
                        ms.append((out_pair[:, t], m2p[:, j, 2, :],
                                   buf[:, j - 1, 1, :]))
                    if j < Tn - 1:
                        ms.append((out_pair[:, t], m2p[:, j, 3, :],
                                   buf[:, j + 1, 3, :]))
                return ms

            def emit_interleaved(la, lb):
                # Alternate between the two accumulation groups (different
                # PSUM banks) so consecutive PE instructions never chain on
                # the same accumulator region.
                na, nb = len(la), len(lb)
                for i in range(max(na, nb)):
                    if i < na:
                        dst, st, mv = la[i]
                        mm(dst, st, mv, start=(i == 0), stop=(i == na - 1))
                    if i < nb:
                        dst, st, mv = lb[i]
                        mm(dst, st, mv, start=(i == 0), stop=(i == nb - 1))

            # Pipeline phases (pair index = it - offset):
            #   A: z/r features        C1: sigmoid        D: h features
            #   B: z/r accumulation    C2: R transpose+    E: h accumulation
            #                              xhr3 update     F: combine + out
            for it in range(NP + 13):
                # ---- A: z/r feature matmuls for pair `it` -----------------
                if it < NP:
                    for t in (0, 1):
                        j = 2 * it + t
                        y = py.tile([128, 4, 128], f32, tag="y")
                        mm(y[:, 0:2, :], xnn[:, j, 0, :], wzr[:, 128:384],
                           start=True, stop=False)
                        mm(y[:, 2:4, :], xnn[:, j, 1, :], wzr[:, 384:640],
                           start=False, stop=True)
                        if t == 0:
                            nc.vector.tensor_copy(scbuf[:, j], y[:])
                        else:
                            nc.scalar.activation(
                                scbuf[:, j], y[:],
                                mybir.ActivationFunctionType.Copy)

                # ---- C2: R transpose + xhr3 H-half update for pair it-6 ---
                if 6 <= it <= NP + 5:
                    p = it - 6
                    ss = s1l[p]
                    tp = pt1.tile([128, 2, 128], f16, tag="t1")
                    mm(tp[:, 0, :], ss[:, 0], ident[:],
                       is_transpose=True, start=True, stop=False)
                    mm(tp[:, 1, :], ss[:, 1], ident[:],
                       is_transpose=True, start=False, stop=True)
                    rts = rts_pool.tile([128, 2, 128], f16, tag="rts")
                    nc.scalar.activation(rts[64:128], tp[64:128],
                                         mybir.ActivationFunctionType.Copy)
                    a = 2 * p
                    rb = rts[64:128, :, None, :].broadcast_to([64, 2, 3, 128])
                    nc.gpsimd.tensor_mul(xhr3[64:128, a:a + 2],
                                         xhr3[64:128, a:a + 2], rb)

                # ---- B+E: z/r accumulation (pair it-3) interleaved with
                #      h accumulation (pair it-12) ---------------------------
                msB, msE, t1 = [], [], None
                if 3 <= it <= NP + 2:
                    p = it - 3
                    t1 = pt1.tile([128, 2, 2, 64], f32, tag="t1")
                    t1l[p] = t1
                    msB = accum_list(t1, scbuf, p, wzr[:, 0:128])
                if 11 <= it <= NP + 10:
                    p = it - 11
                    th = pth.tile([128, 2, 1, 64], f32, tag="th")
                    thl[p] = th
                    msE = accum_list(th, schbuf, p, wh[:, 0:64])
                emit_interleaved(msB, msE)
                if t1 is not None:
                    p = it - 3
                    ss = s1_pool.tile([128, 2, 2, 64], f16, tag="s1")
                    nc.scalar.activation(ss[:], t1[:], SIG)
                    s1l[p] = ss


                # ---- D: h feature matmuls for pair it-8 -------------------
                if 8 <= it <= NP + 7:
                    p = it - 8
                    yh = pyh.tile([128, 2, 4, 64], f32, tag="yh")
                    for t in (0, 1):
                        j = 2 * p + t
                        mm(yh[:, t, 0:2, :], xhr3[:, j, 1, :], wh[:, 64:192],
                           start=(t == 0), stop=False)
                        mm(yh[:, t, 2:4, :], xhr3[:, j, 2, :], wh[:, 192:320],
                           start=False, stop=(t == 1))
                    nc.vector.tensor_copy(schbuf[:, 2 * p:2 * p + 2], yh[:])


                # ---- F: H_tilde + final combine for pair it-12 ------------
                if 12 <= it <= NP + 11:
                    p = it - 12
                    ht = fin_pool.tile([128, 2, 64], f32, tag="ht")
                    nc.scalar.activation(ht[:], thl[p][:, :, 0, :], TANH)
                    d = fin_pool.tile([128, 2, 64], f32, tag="fd")
                    nc.vector.tensor_sub(d[:], hext[:, 2 * p:2 * p + 2, :],
                                         ht[:])
                    e = fin_pool.tile([128, 2, 64], f32, tag="fe")
                    nc.gpsimd.tensor_mul(e[:], d[:], s1l[p][:, :, 0, :])
                    nc.vector.tensor_add(outst[:, 2 * p:2 * p + 2, :], e[:],
                                         ht[:])
                    if (p + 1) % 3 == 0 or p == NP - 1:
                        o0 = (p // 3) * 6
                        o1 = 2 * (p + 1)
                        nc.sync.dma_start(d_out[:, o0:o1], outst[:, o0:o1])

    nc.compile()
    return nc


def _get_program(n_tiles=T):
    if n_tiles not in _prog_cache:
        _prog_cache[n_tiles] = _build_program(n_tiles)
    return _prog_cache[n_tiles]


# ----------------------------------------------------------------------------
# Host-side preparation
# ----------------------------------------------------------------------------
def _band_matrices():
    s = np.arange(128)[:, None]
    d = np.arange(128)[None, :]
    bf_l = ((d - s >= 1) & (d - s <= DEG)).astype(np.float16)
    bf_u = (s - d >= 128 - DEG).astype(np.float16)
    br_l = ((s - d >= 1) & (s - d <= DEG)).astype(np.float16)
    br_u = (d - s >= 128 - DEG).astype(np.float16)
    return np.concatenate([bf_l, bf_u, br_l, br_u], axis=1)  # [128, 512]


def _m2_blocks(no_e, ni_e):
    """Composite level-2 band stationaries, [s, d] layout, per tile.

    M2F_g[d, s] = sum_{m=max(s+1, d-16)}^{min(s+16, d-1)} no[m]  (d-s in [2,32])
    M2R_g[d, s] = sum_{m=max(d+1, s-16)}^{min(d+16, s-1)} ni[m]  (s-d in [2,32])
    Returns [T, 4, 128, 128] f16: M2F_same, M2R_same, M2F_corner (prev
    tile sources, zero-padded to full rows), M2R_corner (next tile).
    """
    n2 = len(no_e)
    Cno = np.concatenate([[0.0], np.cumsum(no_e, dtype=np.float64)])
    Cni = np.concatenate([[0.0], np.cumsum(ni_e, dtype=np.float64)])

    def block(Cm, d, s, rev):
        if rev:
            lo = np.maximum(d + 1, s - 16)
            hi = np.minimum(d + 16, s - 1)
            off = s - d
        else:
            lo = np.maximum(s + 1, d - 16)
            hi = np.minimum(s + 16, d - 1)
            off = d - s
        valid = ((off >= 2) & (off <= 32) & (s >= 0) & (s < n2)
                 & (d >= 0) & (d < n2) & (hi >= lo))
        lo_c = np.clip(lo, 0, n2)
        hi_c = np.clip(hi + 1, 0, n2)
        out = Cm[np.maximum(hi_c, lo_c)] - Cm[lo_c]
        return np.where(valid, out, 0.0).astype(np.float32)

    nt = n2 // 128
    sl = np.arange(128)
    arr = np.zeros((nt, 4, 128, 128), np.float32)
    for t in range(nt):
        t0 = t * 128
        dg = t0 + sl[None, :]           # dest cols
        sg = t0 + sl[:, None]           # source rows (same tile)
        arr[t, 0] = block(Cno, dg, sg, rev=False)
        arr[t, 1] = block(Cni, dg, sg, rev=True)
        if t > 0:
            arr[t, 2] = block(Cno, dg, t0 - 128 + sl[:, None], rev=False)
        if t < nt - 1:
            arr[t, 3] = block(Cni, dg, t0 + 128 + sl[:, None], rev=True)
    return arr.astype(np.float16)


def _pack_weights_zr(Wz, Wr):
    def parts(W):
        w0 = W[0, 0] + W[1, 0] - W[0, 2] - W[1, 2]
        return w0, W[0, 1], 2.0 * W[0, 2], W[1, 1], 2.0 * W[1, 2]
    w0z, f1z, f2z, r1z, r2z = parts(Wz)
    w0r, f1r, f2r, r1r, r2r = parts(Wr)
    return np.concatenate(
        [w0z, w0r, f1z, f1r, f2z, f2r, r1z, r1r, r2z, r2r], axis=1
    ).astype(np.float16)  # [128, 640]


def _pack_weights_h(Wh):
    w0 = Wh[0, 0] + Wh[1, 0] - Wh[0, 2] - Wh[1, 2]
    return np.concatenate(
        [w0, Wh[0, 1], 2.0 * Wh[0, 2], Wh[1, 1], 2.0 * Wh[1, 2]], axis=1
    ).astype(np.float16)  # [128, 320]


def _is_banded_graph(row, col):
    if row.shape != (N * DEG,):
        return False
    r_exp = np.repeat(np.arange(N, dtype=np.int64), DEG)
    if not np.array_equal(row, r_exp):
        return False
    c_exp = (r_exp + np.tile(np.arange(1, DEG + 1, dtype=np.int64), N)) % N
    return np.array_equal(col, c_exp)


def _numpy_fallback(X, H, edge_weight, Wz, bz, Wr, br, Wh, bh, row, col):
    """Exact reference math on the host (only used if inputs deviate)."""
    deg_out = np.bincount(row, weights=edge_weight, minlength=N).astype(np.float32)
    deg_in = np.bincount(col, weights=edge_weight, minlength=N).astype(np.float32)
    norm_out = (1.0 / (deg_out + EPS))[row].astype(np.float32)
    norm_in = (1.0 / (deg_in + EPS))[col].astype(np.float32)

    def prop(x, src, dst, nrm):
        msg = x[:, src, :] * nrm[None, :, None]
        out = np.zeros_like(x)
        np.add.at(out, (slice(None), dst), msg)
        return out

    def dconv(Xc, W, b):
        Hh = Xc @ (W[0, 0] + W[1, 0])
        t1o = prop(Xc, row, col, norm_out)
        t1i = prop(Xc, col, row, norm_in)
        Hh = Hh + t1o @ W[0, 1] + t1i @ W[1, 1]
        for k in range(2, K):
            t1o = 2.0 * prop(t1o, row, col, norm_out) - Xc
            t1i = 2.0 * prop(t1i, col, row, norm_in) - Xc
            Hh = Hh + t1o @ W[0, k] + t1i @ W[1, k]
        return Hh + b

    XH = np.concatenate([X, H], axis=-1)
    Z = 1.0 / (1.0 + np.exp(-dconv(XH, Wz, bz)))
    R = 1.0 / (1.0 + np.exp(-dconv(XH, Wr, br)))
    XHR = np.concatenate([X, H * R], axis=-1)
    Ht = np.tanh(dconv(XHR, Wh, bh))
    return (Z * H + (1.0 - Z) * Ht).astype(np.float32)


def make_in_maps(X, H, edge_weight, Wz, bz, Wr, br, Wh, bh, row, col):
    """Build the 8 per-core input dicts (host sharding + preprocessing)."""
    deg_out = np.bincount(row, weights=edge_weight, minlength=N).astype(np.float32)
    deg_in = np.bincount(col, weights=edge_weight, minlength=N).astype(np.float32)
    n_out = (1.0 / (deg_out + EPS)).astype(np.float32)
    n_in = (1.0 / (deg_in + EPS)).astype(np.float32)

    shared = {
        "wzr": _pack_weights_zr(Wz, Wr),
        "wh": _pack_weights_h(Wh),
        "bands": _band_matrices(),
        "ident": np.eye(128, dtype=np.float16),
    }

    in_maps = []
    for core in range(NCORES):
        b, half = core // 2, core % 2
        g0 = half * HALF - LHALO
        idx = (g0 + np.arange(N2)) % N
        ext = np.concatenate([X[b], H[b]], axis=1)[idx]          # [N2, 128] f32
        no_e = n_out[idx]
        ni_e = n_in[idx]
        m = dict(shared)
        xht = ext.T.astype(np.float16).reshape(128, T, 128)
        xhno = (ext * no_e[:, None]).T.astype(np.float16).reshape(128, T, 128)
        xhni = (ext * ni_e[:, None]).T.astype(np.float16).reshape(128, T, 128)
        m["xnn"] = np.ascontiguousarray(
            np.stack([xhno, xhni], axis=2))                  # [128, T, 2, 128]
        m["xhr3i"] = np.ascontiguousarray(
            np.stack([xht, xhno, xhni], axis=2))             # [128, T, 3, 128]
        m["hext"] = np.ascontiguousarray(
            H[b][idx].reshape(T, 128, 64).transpose(1, 0, 2)).astype(np.float16)
        m["m2p"] = np.ascontiguousarray(
            _m2_blocks(no_e, ni_e).transpose(2, 0, 1, 3))
        in_maps.append(m)
    return in_maps


def unshard_outputs(results):
    out = np.empty((B, N, OUT), dtype=np.float32)
    for core in range(NCORES):
        b, half = core // 2, core % 2
        res = results[core]["out"].astype(np.float32).reshape(128, T, 64)
        ext = res.transpose(1, 0, 2).reshape(N2, 64)
        out[b, half * HALF:(half + 1) * HALF] = ext[LHALO:LHALO + HALF]
    return out


def kernel(X, H, edge_weight, Wz, bz, Wr, br, Wh, bh, edge_index):
    X = np.asarray(X, dtype=np.float32)
    H = np.asarray(H, dtype=np.float32)
    edge_weight = np.asarray(edge_weight, dtype=np.float32)
    Wz = np.asarray(Wz, dtype=np.float32)
    Wr = np.asarray(Wr, dtype=np.float32)
    Wh = np.asarray(Wh, dtype=np.float32)
    bz = np.asarray(bz, dtype=np.float32)
    br = np.asarray(br, dtype=np.float32)
    bh = np.asarray(bh, dtype=np.float32)
    ei = np.asarray(edge_index)
    row = ei[0].astype(np.int64)
    col = ei[1].astype(np.int64)

    if (not _is_banded_graph(row, col) or np.any(bz) or np.any(br)
            or np.any(bh)):
        return _numpy_fallback(X, H, edge_weight, Wz, bz, Wr, br, Wh, bh,
                               row, col)

    from concourse import bass_utils

    nc = _get_program(T)
    in_maps = make_in_maps(X, H, edge_weight, Wz, bz, Wr, br, Wh, bh, row, col)
    res = bass_utils.run_bass_kernel_spmd(nc, in_maps, list(range(NCORES)))
    return unshard_outputs(res.results)
